# revision 14
# baseline (speedup 1.0000x reference)
"""XL-BOMD rank-4 Krylov propagation (EnergyXL) on 8 TRN2 NeuronCores.

Moment-based reformulation: the reference's Gram-Schmidt + rank-4 solve
collapses (exactly, in real arithmetic) to

    out = sum_k delta_k C_k,   C_k = R^k dDS R^k  (pure power sandwiches)

where delta = -L4 @ gamma, G' gamma = c', G'[i][j] = m_{i+j+2},
c'[j] = m_{j+1}, and the B-basis moments m_t come from the C-basis
moments mu_t = <C_i, C_j> (i+j = t, Frobenius) by a forward-difference
(binomial) transform; L4 is the C->B basis binomial matrix.  The
operator v -> R v R is self-adjoint, so mu_t depends only on i+j: nine
inner products total.

Per molecule (N=192): 8 bf16 192^3 matmuls (the only O(N^3) work), 8
PSUM->SBUF copies, 9 Frobenius inner products, and a 4-vector
recombination.  Matrices are stored as [96, 384] tiles (rows 0:96 in
free 0:192, rows 96:192 in free 192:384).  Data-parallel: 64 mols/core,
blocks of 16 share a batched 4x4 symmetric solve on mol-partitions.
"""

import sys

sys.path.insert(0, "/opt/trn_rl_repo")

import numpy as np
import ml_dtypes

import concourse.bass as bass
import concourse.bacc as bacc
import concourse.tile as tile
from concourse import mybir
from concourse.bass_utils import run_bass_kernel_spmd

F32 = mybir.dt.float32
BF16 = mybir.dt.bfloat16
ALU = mybir.AluOpType
ACTF = mybir.ActivationFunctionType

NMOL, N, RANK = 512, 192, 4
NCORES = 8
MPC = NMOL // NCORES      # 64 molecules per core
H, F = 96, 384            # [96, 384] tile layout for a 192x192 matrix
BLK = 16                  # molecules per solve block

# mu_t = <C_i, C_j> pairing per moment column t (i + j = t)
MUPAIR = [(0, 0), (0, 1), (1, 1), (1, 2), (2, 2), (2, 3), (3, 3), (3, 4),
          (4, 4)]
# reduce-engine per moment: diag on ACT = fused Square+accum (1 op);
# off-diag muls always DVE, reduce spread across engines for balance
MOM_RED = {0: "act", 1: "act_acc", 2: "act", 3: "dve", 4: "act", 5: "dve",
           6: "act", 7: "dve", 8: "act"}
# psum->sbuf copy engines: 4 T-copies then 4 C-copies
ENG_TCOPY = ["act", "act", "act", "dve"]
ENG_CCOPY = ["act", "dve", "dve", "dve"]
# forward-difference table offsets: V_t occupies cols OFS[t] .. OFS[t]+(9-t)
OFS = [0, 9, 17, 24, 30, 35, 39, 42, 44]


def _mcol(t):
    """Column of m_t (= first entry of V_t) in the W difference tile."""
    return OFS[t]


def build_core_kernel(n_mols=MPC):
    nc = bacc.Bacc(None, target_bir_lowering=False, enable_partition_id=False)
    D = nc.dram_tensor("D", [n_mols, N, N], BF16, kind="ExternalInput")
    P = nc.dram_tensor("P", [n_mols, N, N], BF16, kind="ExternalInput")
    R = nc.dram_tensor("Rm", [n_mols, N, N], BF16, kind="ExternalInput")
    OUT = nc.dram_tensor("OUT", [n_mols, N, N], F32, kind="ExternalOutput")
    with tile.TileContext(nc) as tc:
        _body(nc, tc, D, P, R, OUT)
    nc.finalize()
    return nc


def _mm_sandwich(nc, ps, lhsT, rhs):
    """ps[96,384] (psum) = lhsT^T @ rhs for 192x192 operands in [96,384]
    layout. lhsT must be symmetric-as-stored (we always pass symmetric
    matrices as lhsT)."""
    nc.tensor.matmul(ps[:, 0:192], lhsT=lhsT[:, 0:96], rhs=rhs[:, 0:192],
                     start=True, stop=False)
    nc.tensor.matmul(ps[:, 0:192], lhsT=lhsT[:, 192:288], rhs=rhs[:, 192:384],
                     start=False, stop=True)
    nc.tensor.matmul(ps[:, 192:384], lhsT=lhsT[:, 96:192], rhs=rhs[:, 0:192],
                     start=True, stop=False)
    nc.tensor.matmul(ps[:, 192:384], lhsT=lhsT[:, 288:384],
                     rhs=rhs[:, 192:384], start=False, stop=True)


def _body(nc, tc, D, P, R, OUT):
    import contextlib

    ctx = contextlib.ExitStack()
    with ctx:
        consts = ctx.enter_context(tc.tile_pool(name="consts", bufs=1))
        inp = ctx.enter_context(tc.tile_pool(name="inp", bufs=6))
        cper = ctx.enter_context(tc.tile_pool(name="cper", bufs=BLK + 5))
        cshort = ctx.enter_context(tc.tile_pool(name="cshort", bufs=6))
        junk = ctx.enter_context(tc.tile_pool(name="junk", bufs=6))
        parts = ctx.enter_context(tc.tile_pool(name="parts", bufs=BLK + 5))
        comb = ctx.enter_context(tc.tile_pool(name="comb", bufs=4))
        blkp = ctx.enter_context(tc.tile_pool(name="blkp", bufs=2))
        ps_mm = ctx.enter_context(tc.tile_pool(name="ps_mm", bufs=3,
                                               space="PSUM"))
        ps_g = ctx.enter_context(tc.tile_pool(name="ps_g", bufs=1,
                                              space="PSUM"))
        ps_bc = ctx.enter_context(tc.tile_pool(name="ps_bc", bufs=1,
                                               space="PSUM"))

        # --- constants ---
        sel = consts.tile([H, 2 * BLK - 1], F32)   # windowed one-hot column
        nc.vector.memset(sel, 0.0)
        nc.vector.memset(sel[:, BLK - 1 : BLK], 1.0)
        ones = consts.tile([H, H], F32)            # bcast lhsT (rows 0:16)
        nc.vector.memset(ones, 1.0)
        id16 = consts.tile([BLK, BLK], F32)
        idt = consts.tile([BLK, BLK], mybir.dt.int32)
        nc.gpsimd.iota(idt, pattern=[[-1, BLK]], base=0, channel_multiplier=1)
        nc.vector.tensor_scalar(out=id16, in0=idt, scalar1=0, scalar2=None,
                                op0=ALU.is_equal)
        i96 = consts.tile([H, H], BF16)            # identity, combo lhsT seed
        idt96 = consts.tile([H, H], mybir.dt.int32)
        nc.gpsimd.iota(idt96, pattern=[[-1, H]], base=0, channel_multiplier=1)
        nc.vector.tensor_scalar(out=i96, in0=idt96, scalar1=0, scalar2=None,
                                op0=ALU.is_equal)

        n_mols = D.shape[0]
        for b in range(n_mols // BLK):
            mols = list(range(b * BLK, (b + 1) * BLK))
            st = [_mol_chain(nc, D, P, R, m, inp, cper, cshort, junk, parts,
                             ps_mm) for m in mols]
            dbc = _block_tail(nc, b, st, consts, blkp, ps_g, ps_bc, sel, ones,
                              id16)
            for j, (m, s) in enumerate(zip(mols, st)):
                _combo(nc, OUT, m, j, s, dbc, comb, ps_mm, i96)


def _mol_chain(nc, D, P, R, m, inp, cper, cshort, junk, parts, ps_mm):
    """Emit one molecule's power chain + moment accumulations."""
    d_t = inp.tile([H, F], BF16, tag="d_in")
    p_t = inp.tile([H, F], BF16, tag="p_in")
    r_t = inp.tile([H, F], BF16, tag="r_in")
    for tile_, src in ((d_t, D), (p_t, P), (r_t, R)):
        nc.sync.dma_start(out=tile_[:, 0:192], in_=src[m, 0:H, :])
        nc.sync.dma_start(out=tile_[:, 192:384], in_=src[m, H:N, :])

    part = parts.tile([H, 12], F32, tag="part")

    # C0 = D - P (bf16, DVE 2x)
    c = [None] * (RANK + 1)
    c[0] = cper.tile([H, F], BF16, tag="c0", name="c0")
    nc.vector.tensor_sub(c[0], d_t, p_t)

    for i in range(1, RANK + 1):
        # T = C_{i-1} R
        psT = ps_mm.tile([H, F], F32, tag="pT")
        _mm_sandwich(nc, psT, c[i - 1], r_t)
        t_t = cshort.tile([H, F], BF16, tag="t")
        if ENG_TCOPY[i - 1] == "act":
            nc.scalar.copy(t_t, psT)
        else:
            nc.vector.tensor_copy(t_t, psT)
        # C_i = R T
        psC = ps_mm.tile([H, F], F32, tag="pC")
        _mm_sandwich(nc, psC, r_t, t_t)
        pool = cper if i < RANK else cshort
        c[i] = pool.tile([H, F], BF16, tag=f"c{i}", name=f"c{i}")
        if ENG_CCOPY[i - 1] == "act":
            nc.scalar.copy(c[i], psC)
        else:
            nc.vector.tensor_copy(c[i], psC)

    # moments mu_t -> part[:, t]
    # diag (i==j): one ACT Square+accum.  off-diag: DVE mul (bf16 2x) into
    # junk scratch, then a reduce on the engine given by MOM_RED.
    for t, (i, j) in enumerate(MUPAIR):
        if i == j and MOM_RED[t] == "act":
            ja = junk.tile([H, F], BF16, tag="junk_a")
            nc.scalar.activation(out=ja, in_=c[i], func=ACTF.Square,
                                 accum_out=part[:, t : t + 1])
            continue
        jd = junk.tile([H, F], BF16, tag="junk_d")
        nc.vector.tensor_mul(jd, c[i], c[j])
        red = MOM_RED[t]
        if red == "act_acc":
            ja2 = junk.tile([H, F], BF16, tag="junk_a2")
            nc.scalar.activation(out=ja2, in_=jd, func=ACTF.Copy,
                                 accum_out=part[:, t : t + 1])
        else:
            nc.vector.tensor_reduce(out=part[:, t : t + 1], in_=jd,
                                    axis=mybir.AxisListType.X, op=ALU.add)

    return {"part": part, "c": c[: RANK]}


def _block_tail(nc, b, st, consts, blkp, ps_g, ps_bc, sel, ones, id16):
    """Gather moments, difference-transform, solve, broadcast deltas."""
    gps = ps_g.tile([BLK, 9], F32, tag="g")
    for j, s in enumerate(st):
        nc.tensor.matmul(gps, lhsT=sel[:, BLK - 1 - j : 2 * BLK - 1 - j],
                         rhs=s["part"][:, 0:9], start=(j == 0),
                         stop=(j == len(st) - 1))

    W = blkp.tile([BLK, 45], F32, tag="W")
    nc.vector.tensor_copy(W[:, 0:9], gps)
    # forward differences: V_t[k] = V_{t-1}[k] - V_{t-1}[k-1], k = t..8
    for t in range(1, 9):
        w = 9 - t
        o, po = OFS[t], OFS[t - 1]
        nc.vector.tensor_sub(W[:, o : o + w], W[:, po + 1 : po + 1 + w],
                             W[:, po : po + w])

    # private copies of solver-overwritten inputs
    S = blkp.tile([BLK, 10], F32, tag="S")
    priv = [4, 5, 6, 6, 7, 8, 1, 2, 3, 4]  # e f h gg i_ jj r0... see below
    # S cols: 0:e<-m4 1:f<-m5 2:h<-m6 3:gg<-m6 4:i_<-m7 5:jj<-m8
    #         6:r0<-m1 7:r1<-m2 8:r2<-m3 9:r3<-m4
    for scol, mt in zip(range(10), (4, 5, 6, 6, 7, 8, 1, 2, 3, 4)):
        nc.vector.tensor_copy(S[:, scol : scol + 1],
                              W[:, _mcol(mt) : _mcol(mt) + 1])

    X = blkp.tile([BLK, 16], F32, tag="X")
    ys = _solve_sym4(nc, W, S, X)

    # delta = -L4 @ gamma  (C-basis output coefficients)
    DL = blkp.tile([BLK, 4], F32, tag="DL")
    y0, y1, y2, y3 = ys
    ta = X[:, 12:13]
    u = X[:, 13:14]
    v = X[:, 14:15]
    w2 = X[:, 15:16]
    mul = nc.vector.tensor_mul
    sub = nc.vector.tensor_sub
    add = nc.vector.tensor_add
    ts = nc.vector.tensor_scalar
    ts(out=DL[:, 3:4], in0=y3, scalar1=-1.0, scalar2=None, op0=ALU.mult)
    ts(out=ta, in0=y3, scalar1=3.0, scalar2=None, op0=ALU.mult)
    sub(DL[:, 2:3], ta, y2)                      # 3*y3 - y2
    ts(out=u, in0=y2, scalar1=2.0, scalar2=None, op0=ALU.mult)
    sub(v, u, y1)                                # 2*y2 - y1
    sub(DL[:, 1:2], v, ta)                       # 2*y2 - y1 - 3*y3
    sub(w2, y1, y0)
    sub(u, y3, y2)
    add(DL[:, 0:1], w2, u)                       # y1 - y0 + y3 - y2

    # broadcast deltas: dbc[:, 4j+k] = delta_k of mol j, on 96 partitions
    bcps = ps_bc.tile([H, 4 * BLK], F32, tag="bc")
    for j in range(BLK):
        mk = blkp.tile([BLK, 4], F32, tag="mk")
        ts(out=mk, in0=DL, scalar1=id16[:, j : j + 1], scalar2=None,
           op0=ALU.mult)
        nc.tensor.matmul(bcps[:, 4 * j : 4 * j + 4], lhsT=ones[0:BLK, 0:H],
                         rhs=mk, start=True, stop=True)
    dbc = blkp.tile([H, 4 * BLK], F32, tag="dbc")
    nc.scalar.copy(dbc, bcps)
    return dbc


def _combo(nc, OUT, m, j, s, dbc, comb, ps_mm, i96):
    """out = sum_k delta_k C_k via PE accumulation of (delta_k I) @ C_k."""
    c = s["c"]
    ts = nc.vector.tensor_scalar
    dg = [comb.tile([H, H], BF16, tag=f"dg{k}", name=f"dg{k}")
          for k in range(4)]
    for k in range(4):
        ts(out=dg[k], in0=i96, scalar1=dbc[:, 4 * j + k : 4 * j + k + 1],
           scalar2=None, op0=ALU.mult)
    pso = ps_mm.tile([H, F], F32, tag="pT")
    for k in range(4):
        nc.tensor.matmul(pso, lhsT=dg[k], rhs=c[k], start=(k == 0),
                         stop=(k == 3))
    os_ = comb.tile([H, F], F32, tag="os")
    nc.scalar.copy(os_, pso)
    nc.sync.dma_start(out=OUT[m, 0:H, :], in_=os_[:, 0:192])
    nc.sync.dma_start(out=OUT[m, H:N, :], in_=os_[:, 192:384])


def _solve_sym4(nc, W, S, X):
    """Batched 4x4 symmetric solve on [BLK,1] column APs.

    Hankel inputs: O_ij = m_{i+j+2} (views into W), rhs c_j = m_{j+1}.
    Overwritten entries live in S (private copies); X is scratch.
    Returns [y0..y3] column APs (in X cols 8..11).
    Mirrors _solve_sym4_np below; keep in sync."""
    def wm(t):
        return W[:, _mcol(t) : _mcol(t) + 1]

    def sc(i):
        return S[:, i : i + 1]

    a, bb, cc, dd = wm(2), wm(3), wm(4), wm(5)       # read-only
    e, f, h, gg, i_, jj = (sc(k) for k in range(6))  # overwritten
    r0, r1, r2, r3 = (sc(k) for k in range(6, 10))
    p0, p1, p2, p3 = (X[:, k : k + 1] for k in range(4))
    l1, l2, l3 = (X[:, k : k + 1] for k in range(4, 7))
    t0 = X[:, 7:8]
    y0, y1, y2, y3 = (X[:, k : k + 1] for k in range(8, 12))

    mul = nc.vector.tensor_mul
    sub = nc.vector.tensor_sub
    rec = nc.vector.reciprocal

    def upd(x, l, src):  # x -= l*src
        mul(t0, l, src)
        sub(x, x, t0)

    rec(p0, a)
    mul(l1, bb, p0); mul(l2, cc, p0); mul(l3, dd, p0)
    upd(e, l1, bb); upd(f, l2, bb); upd(gg, l3, bb)
    upd(h, l2, cc); upd(i_, l3, cc); upd(jj, l3, dd)
    upd(r1, l1, r0); upd(r2, l2, r0); upd(r3, l3, r0)

    rec(p1, e)
    mul(l2, f, p1); mul(l3, gg, p1)
    upd(h, l2, f); upd(i_, l3, f); upd(jj, l3, gg)
    upd(r2, l2, r1); upd(r3, l3, r1)

    rec(p2, h)
    mul(l3, i_, p2)
    upd(jj, l3, i_); upd(r3, l3, r2)

    rec(p3, jj)
    mul(y3, r3, p3)
    upd(r2, i_, y3); mul(y2, r2, p2)
    upd(r1, f, y2); upd(r1, gg, y3); mul(y1, r1, p1)
    upd(r0, bb, y1); upd(r0, cc, y2); upd(r0, dd, y3); mul(y0, r0, p0)
    return [y0, y1, y2, y3]


# ---------------------------------------------------------------------------
# numpy mirror (for verification without hardware)

def _bf(x):
    return np.asarray(x).astype(ml_dtypes.bfloat16).astype(np.float32)


def _solve_sym4_np(m):
    """m: [n, 9] float32 (m[:, t] = m_t, col 0 unused). Returns y [n, 4]."""
    col = lambda t: m[:, t : t + 1].astype(np.float32)
    a, bb, cc, dd = col(2), col(3), col(4), col(5)
    e, f, h, gg, i_, jj = col(4), col(5), col(6), col(6), col(7), col(8)
    r0, r1, r2, r3 = col(1), col(2), col(3), col(4)
    p0 = np.float32(1.0) / a
    l1, l2, l3 = bb * p0, cc * p0, dd * p0
    e = e - l1 * bb; f = f - l2 * bb; gg = gg - l3 * bb
    h = h - l2 * cc; i_ = i_ - l3 * cc; jj = jj - l3 * dd
    r1 = r1 - l1 * r0; r2 = r2 - l2 * r0; r3 = r3 - l3 * r0
    p1 = np.float32(1.0) / e
    l2, l3 = f * p1, gg * p1
    h = h - l2 * f; i_ = i_ - l3 * f; jj = jj - l3 * gg
    r2 = r2 - l2 * r1; r3 = r3 - l3 * r1
    p2 = np.float32(1.0) / h
    l3 = i_ * p2
    jj = jj - l3 * i_; r3 = r3 - l3 * r2
    p3 = np.float32(1.0) / jj
    y3 = r3 * p3
    r2 = r2 - i_ * y3; y2 = r2 * p2
    r1 = r1 - f * y2; r1 = r1 - gg * y3; y1 = r1 * p1
    r0 = r0 - bb * y1; r0 = r0 - cc * y2; r0 = r0 - dd * y3; y0 = r0 * p0
    return np.concatenate([y0, y1, y2, y3], axis=1)


def _mirror_numpy(D, P, R):
    """Bit-approximate mirror of the device algorithm (bf16 rounding at the
    same points), for offline validation."""
    Db, Pb, Rb = _bf(D), _bf(P), _bf(R)
    b = D.shape[0]
    C = [None] * (RANK + 1)
    C[0] = _bf(Db - Pb)
    for i in range(1, RANK + 1):
        T = _bf(np.einsum("bij,bjk->bik", C[i - 1], Rb, dtype=np.float32))
        C[i] = _bf(np.einsum("bij,bjk->bik", Rb, T, dtype=np.float32))
    mu = np.zeros((b, 9), dtype=np.float32)
    for t, (i, j) in enumerate(MUPAIR):
        mu[:, t] = np.sum(C[i].astype(np.float32) * C[j].astype(np.float32),
                          axis=(1, 2))
    # forward differences
    V = mu.copy()
    m = np.zeros((b, 9), dtype=np.float32)
    for t in range(1, 9):
        V = (V[:, 1:] - V[:, :-1]).astype(np.float32)
        m[:, t] = V[:, 0]
    y = _solve_sym4_np(m)
    y0, y1, y2, y3 = (y[:, k : k + 1] for k in range(4))
    d3 = -y3
    d2 = 3 * y3 - y2
    d1 = 2 * y2 - y1 - 3 * y3
    d0 = y1 - y0 + y3 - y2
    dl = _bf(np.concatenate([d0, d1, d2, d3], axis=1))
    # PE combo: (delta_k I)_bf16 @ C_k accumulated in fp32 PSUM
    return sum(dl[:, k, None, None] * C[k] for k in range(4)).astype(np.float32)


# ---------------------------------------------------------------------------

_NC_CACHE = None


def _get_nc():
    global _NC_CACHE
    if _NC_CACHE is None:
        _NC_CACHE = build_core_kernel()
    return _NC_CACHE


def kernel(D, P, R, max_rank=4, _trace=False):
    BF = ml_dtypes.bfloat16
    D = np.ascontiguousarray(np.asarray(D, dtype=np.float32).astype(BF))
    P = np.ascontiguousarray(np.asarray(P, dtype=np.float32).astype(BF))
    R = np.ascontiguousarray(np.asarray(R, dtype=np.float32).astype(BF))
    nc = _get_nc()
    in_maps = []
    for i in range(NCORES):
        sl = slice(i * MPC, (i + 1) * MPC)
        in_maps.append({"D": D[sl], "P": P[sl], "Rm": R[sl]})
    res = run_bass_kernel_spmd(nc, in_maps, core_ids=list(range(NCORES)),
                               trace=_trace)
    out = np.concatenate([r["OUT"] for r in res.results], axis=0)
    if _trace:
        kernel.last_exec_time_ns = res.exec_time_ns
        kernel.last_trace = res.instructions_and_trace
    return out


if __name__ == "__main__":
    # offline mirror check against the jax reference
    sys.path.insert(0, "/root/problem")
    import jax

    jax.config.update("jax_platforms", "cpu")
    import reference

    inputs = {k: np.asarray(v) for k, v in reference.setup_inputs().items()}
    expected = np.asarray(reference.reference(**reference.setup_inputs()))
    got = _mirror_numpy(inputs["D"], inputs["P"], inputs["R"])
    scale = np.abs(expected).max()
    rel = np.abs(got - expected).max() / scale
    print(f"mirror rel err: {rel:.3e} (scale {scale:.3f})")


# revision 15
# speedup vs baseline: 1.0751x; 1.0751x over previous
"""XL-BOMD rank-4 Krylov propagation (EnergyXL) on 8 TRN2 NeuronCores.

Moment-based reformulation: the reference's Gram-Schmidt + rank-4 solve
collapses (exactly, in real arithmetic) to

    out = sum_k delta_k C_k,   C_k = R^k dDS R^k  (pure power sandwiches)

where delta = -L4 @ gamma, G' gamma = c', G'[i][j] = m_{i+j+2},
c'[j] = m_{j+1}, and the B-basis moments m_t come from the C-basis
moments mu_t = <C_i, C_j> (i+j = t, Frobenius) by a forward-difference
(binomial) transform; L4 is the C->B basis binomial matrix.  The
operator v -> R v R is self-adjoint, so mu_t depends only on i+j: nine
inner products total.

Per molecule (N=192): 8 bf16 192^3 matmuls (the only O(N^3) work), 8
PSUM->SBUF copies, 9 Frobenius inner products, and a 4-vector
recombination.  Matrices are stored as [96, 384] tiles (rows 0:96 in
free 0:192, rows 96:192 in free 192:384).  Data-parallel: 64 mols/core,
blocks of 16 share a batched 4x4 symmetric solve on mol-partitions.
"""

import sys

sys.path.insert(0, "/opt/trn_rl_repo")

import numpy as np
import ml_dtypes

import concourse.bass as bass
import concourse.bacc as bacc
import concourse.tile as tile
from concourse import mybir
from concourse.bass_utils import run_bass_kernel_spmd

F32 = mybir.dt.float32
BF16 = mybir.dt.bfloat16
ALU = mybir.AluOpType
ACTF = mybir.ActivationFunctionType

NMOL, N, RANK = 512, 192, 4
NCORES = 8
MPC = NMOL // NCORES      # 64 molecules per core
H, F = 96, 384            # [96, 384] tile layout for a 192x192 matrix
BLK = 16                  # molecules per solve block

# mu_t = <C_i, C_j> pairing per moment column t (i + j = t)
MUPAIR = [(0, 0), (0, 1), (1, 1), (1, 2), (2, 2), (2, 3), (3, 3), (3, 4),
          (4, 4)]
# reduce-engine per moment: diag on ACT = fused Square+accum (1 op);
# off-diag muls always DVE, reduce spread across engines for balance
MOM_RED = {0: "act", 1: "act_acc", 2: "act", 3: "dve", 4: "act", 5: "dve",
           6: "act", 7: "dve", 8: "act"}
# psum->sbuf copy engines: 4 T-copies then 4 C-copies
ENG_TCOPY = ["act", "act", "act", "dve"]
ENG_CCOPY = ["act", "dve", "dve", "dve"]
# forward-difference table offsets: V_t occupies cols OFS[t] .. OFS[t]+(9-t)
OFS = [0, 9, 17, 24, 30, 35, 39, 42, 44]


def _mcol(t):
    """Column of m_t (= first entry of V_t) in the W difference tile."""
    return OFS[t]


def build_core_kernel(n_mols=MPC):
    nc = bacc.Bacc(None, target_bir_lowering=False, enable_partition_id=False)
    D = nc.dram_tensor("D", [n_mols, N, N], BF16, kind="ExternalInput")
    P = nc.dram_tensor("P", [n_mols, N, N], BF16, kind="ExternalInput")
    R = nc.dram_tensor("Rm", [n_mols, N, N], BF16, kind="ExternalInput")
    OUT = nc.dram_tensor("OUT", [n_mols, N, N], F32, kind="ExternalOutput")
    with tile.TileContext(nc) as tc:
        _body(nc, tc, D, P, R, OUT)
    nc.finalize()
    return nc


def _mm_sandwich(nc, ps, lhsT, rhs):
    """ps[96,384] (psum) = lhsT^T @ rhs for 192x192 operands in [96,384]
    layout. lhsT must be symmetric-as-stored (we always pass symmetric
    matrices as lhsT)."""
    nc.tensor.matmul(ps[:, 0:192], lhsT=lhsT[:, 0:96], rhs=rhs[:, 0:192],
                     start=True, stop=False)
    nc.tensor.matmul(ps[:, 0:192], lhsT=lhsT[:, 192:288], rhs=rhs[:, 192:384],
                     start=False, stop=True)
    nc.tensor.matmul(ps[:, 192:384], lhsT=lhsT[:, 96:192], rhs=rhs[:, 0:192],
                     start=True, stop=False)
    nc.tensor.matmul(ps[:, 192:384], lhsT=lhsT[:, 288:384],
                     rhs=rhs[:, 192:384], start=False, stop=True)


def _body(nc, tc, D, P, R, OUT):
    import contextlib

    ctx = contextlib.ExitStack()
    with ctx:
        consts = ctx.enter_context(tc.tile_pool(name="consts", bufs=1))
        inp = ctx.enter_context(tc.tile_pool(name="inp", bufs=8))
        cper = ctx.enter_context(tc.tile_pool(name="cper", bufs=BLK + 5))
        cshort = ctx.enter_context(tc.tile_pool(name="cshort", bufs=6))
        junk = ctx.enter_context(tc.tile_pool(name="junk", bufs=6))
        parts = ctx.enter_context(tc.tile_pool(name="parts", bufs=BLK + 5))
        comb = ctx.enter_context(tc.tile_pool(name="comb", bufs=4))
        blkp = ctx.enter_context(tc.tile_pool(name="blkp", bufs=2))
        ps_mm = ctx.enter_context(tc.tile_pool(name="ps_mm", bufs=2,
                                               space="PSUM"))
        ps_o = ctx.enter_context(tc.tile_pool(name="ps_o", bufs=1,
                                              space="PSUM"))
        ps_g = ctx.enter_context(tc.tile_pool(name="ps_g", bufs=1,
                                              space="PSUM"))
        ps_bc = ctx.enter_context(tc.tile_pool(name="ps_bc", bufs=1,
                                               space="PSUM"))

        # --- constants ---
        sel = consts.tile([H, 2 * BLK - 1], F32)   # windowed one-hot column
        nc.vector.memset(sel, 0.0)
        nc.vector.memset(sel[:, BLK - 1 : BLK], 1.0)
        ones = consts.tile([H, H], F32)            # bcast lhsT (rows 0:16)
        nc.vector.memset(ones, 1.0)
        id16 = consts.tile([BLK, BLK], F32)
        idt = consts.tile([BLK, BLK], mybir.dt.int32)
        nc.gpsimd.iota(idt, pattern=[[-1, BLK]], base=0, channel_multiplier=1)
        nc.vector.tensor_scalar(out=id16, in0=idt, scalar1=0, scalar2=None,
                                op0=ALU.is_equal)
        i96 = consts.tile([H, H], BF16)            # identity, combo lhsT seed
        idt96 = consts.tile([H, H], mybir.dt.int32)
        nc.gpsimd.iota(idt96, pattern=[[-1, H]], base=0, channel_multiplier=1)
        nc.vector.tensor_scalar(out=i96, in0=idt96, scalar1=0, scalar2=None,
                                op0=ALU.is_equal)

        n_mols = D.shape[0]
        for b in range(n_mols // BLK):
            mols = list(range(b * BLK, (b + 1) * BLK))
            st = [_mol_chain(nc, tc, D, P, R, m, inp, cper, cshort, junk,
                             parts, ps_mm) for m in mols]
            dbc = _block_tail(nc, b, st, consts, blkp, ps_g, ps_bc, sel, ones,
                              id16)
            for j, (m, s) in enumerate(zip(mols, st)):
                _combo(nc, OUT, m, j, s, dbc, comb, ps_o, i96)


def _mol_chain(nc, tc, D, P, R, m, inp, cper, cshort, junk, parts, ps_mm):
    """Emit one molecule's power chain + moment accumulations."""
    d_t = inp.tile([H, F], BF16, tag="d_in")
    p_t = inp.tile([H, F], BF16, tag="p_in")
    r_t = inp.tile([H, F], BF16, tag="r_in")
    for tile_, src in ((d_t, D), (p_t, P), (r_t, R)):
        nc.sync.dma_start(out=tile_[:, 0:192], in_=src[m, 0:H, :])
        nc.sync.dma_start(out=tile_[:, 192:384], in_=src[m, H:N, :])

    part = parts.tile([H, 12], F32, tag="part")

    # C0 = D - P (bf16, DVE 2x)
    c = [None] * (RANK + 1)
    c[0] = cper.tile([H, F], BF16, tag="c0", name="c0")
    nc.vector.tensor_sub(c[0], d_t, p_t)

    for i in range(1, RANK + 1):
        # T = C_{i-1} R
        psT = ps_mm.tile([H, F], F32, tag="pT", bufs=3)
        _mm_sandwich(nc, psT, c[i - 1], r_t)
        t_t = cshort.tile([H, F], BF16, tag="t")
        with tc.high_priority(offset=300):
            if ENG_TCOPY[i - 1] == "act":
                nc.scalar.copy(t_t, psT)
            else:
                nc.vector.tensor_copy(t_t, psT)
        # C_i = R T
        psC = ps_mm.tile([H, F], F32, tag="pC")
        _mm_sandwich(nc, psC, r_t, t_t)
        pool = cper if i < RANK else cshort
        c[i] = pool.tile([H, F], BF16, tag=f"c{i}", name=f"c{i}")
        with tc.high_priority(offset=300):
            if ENG_CCOPY[i - 1] == "act":
                nc.scalar.copy(c[i], psC)
            else:
                nc.vector.tensor_copy(c[i], psC)

    # moments mu_t -> part[:, t]
    # diag (i==j): one ACT Square+accum.  off-diag: DVE mul (bf16 2x) into
    # junk scratch, then a reduce on the engine given by MOM_RED.
    for t, (i, j) in enumerate(MUPAIR):
        if i == j and MOM_RED[t] == "act":
            ja = junk.tile([H, F], BF16, tag="junk_a")
            nc.scalar.activation(out=ja, in_=c[i], func=ACTF.Square,
                                 accum_out=part[:, t : t + 1])
            continue
        jd = junk.tile([H, F], BF16, tag="junk_d")
        nc.vector.tensor_mul(jd, c[i], c[j])
        red = MOM_RED[t]
        if red == "act_acc":
            ja2 = junk.tile([H, F], BF16, tag="junk_a2")
            nc.scalar.activation(out=ja2, in_=jd, func=ACTF.Copy,
                                 accum_out=part[:, t : t + 1])
        else:
            nc.vector.tensor_reduce(out=part[:, t : t + 1], in_=jd,
                                    axis=mybir.AxisListType.X, op=ALU.add)

    return {"part": part, "c": c[: RANK]}


def _block_tail(nc, b, st, consts, blkp, ps_g, ps_bc, sel, ones, id16):
    """Gather moments, difference-transform, solve, broadcast deltas."""
    gps = ps_g.tile([BLK, 9], F32, tag="g")
    for j, s in enumerate(st):
        nc.tensor.matmul(gps, lhsT=sel[:, BLK - 1 - j : 2 * BLK - 1 - j],
                         rhs=s["part"][:, 0:9], start=(j == 0),
                         stop=(j == len(st) - 1))

    W = blkp.tile([BLK, 45], F32, tag="W")
    nc.vector.tensor_copy(W[:, 0:9], gps)
    # forward differences: V_t[k] = V_{t-1}[k] - V_{t-1}[k-1], k = t..8
    for t in range(1, 9):
        w = 9 - t
        o, po = OFS[t], OFS[t - 1]
        nc.vector.tensor_sub(W[:, o : o + w], W[:, po + 1 : po + 1 + w],
                             W[:, po : po + w])

    # private copies of solver-overwritten inputs
    S = blkp.tile([BLK, 10], F32, tag="S")
    priv = [4, 5, 6, 6, 7, 8, 1, 2, 3, 4]  # e f h gg i_ jj r0... see below
    # S cols: 0:e<-m4 1:f<-m5 2:h<-m6 3:gg<-m6 4:i_<-m7 5:jj<-m8
    #         6:r0<-m1 7:r1<-m2 8:r2<-m3 9:r3<-m4
    for scol, mt in zip(range(10), (4, 5, 6, 6, 7, 8, 1, 2, 3, 4)):
        nc.vector.tensor_copy(S[:, scol : scol + 1],
                              W[:, _mcol(mt) : _mcol(mt) + 1])

    X = blkp.tile([BLK, 16], F32, tag="X")
    ys = _solve_sym4(nc, W, S, X)

    # delta = -L4 @ gamma  (C-basis output coefficients)
    DL = blkp.tile([BLK, 4], F32, tag="DL")
    y0, y1, y2, y3 = ys
    ta = X[:, 12:13]
    u = X[:, 13:14]
    v = X[:, 14:15]
    w2 = X[:, 15:16]
    mul = nc.vector.tensor_mul
    sub = nc.vector.tensor_sub
    add = nc.vector.tensor_add
    ts = nc.vector.tensor_scalar
    ts(out=DL[:, 3:4], in0=y3, scalar1=-1.0, scalar2=None, op0=ALU.mult)
    ts(out=ta, in0=y3, scalar1=3.0, scalar2=None, op0=ALU.mult)
    sub(DL[:, 2:3], ta, y2)                      # 3*y3 - y2
    ts(out=u, in0=y2, scalar1=2.0, scalar2=None, op0=ALU.mult)
    sub(v, u, y1)                                # 2*y2 - y1
    sub(DL[:, 1:2], v, ta)                       # 2*y2 - y1 - 3*y3
    sub(w2, y1, y0)
    sub(u, y3, y2)
    add(DL[:, 0:1], w2, u)                       # y1 - y0 + y3 - y2

    # broadcast deltas: dbc[:, 4j+k] = delta_k of mol j, on 96 partitions
    bcps = ps_bc.tile([H, 4 * BLK], F32, tag="bc")
    for j in range(BLK):
        mk = blkp.tile([BLK, 4], F32, tag="mk")
        ts(out=mk, in0=DL, scalar1=id16[:, j : j + 1], scalar2=None,
           op0=ALU.mult)
        nc.tensor.matmul(bcps[:, 4 * j : 4 * j + 4], lhsT=ones[0:BLK, 0:H],
                         rhs=mk, start=True, stop=True)
    dbc = blkp.tile([H, 4 * BLK], F32, tag="dbc")
    nc.scalar.copy(dbc, bcps)
    return dbc


def _combo(nc, OUT, m, j, s, dbc, comb, ps_o, i96):
    """out = sum_k delta_k C_k via PE accumulation of (delta_k I) @ C_k."""
    c = s["c"]
    ts = nc.vector.tensor_scalar
    dg = [comb.tile([H, H], BF16, tag=f"dg{k}", name=f"dg{k}")
          for k in range(4)]
    for k in range(4):
        ts(out=dg[k], in0=i96, scalar1=dbc[:, 4 * j + k : 4 * j + k + 1],
           scalar2=None, op0=ALU.mult)
    pso = ps_o.tile([H, F], F32, tag="po")
    for k in range(4):
        nc.tensor.matmul(pso, lhsT=dg[k], rhs=c[k], start=(k == 0),
                         stop=(k == 3))
    os_ = comb.tile([H, F], F32, tag="os")
    nc.scalar.copy(os_, pso)
    nc.sync.dma_start(out=OUT[m, 0:H, :], in_=os_[:, 0:192])
    nc.sync.dma_start(out=OUT[m, H:N, :], in_=os_[:, 192:384])


def _solve_sym4(nc, W, S, X):
    """Batched 4x4 symmetric solve on [BLK,1] column APs.

    Hankel inputs: O_ij = m_{i+j+2} (views into W), rhs c_j = m_{j+1}.
    Overwritten entries live in S (private copies); X is scratch.
    Returns [y0..y3] column APs (in X cols 8..11).
    Mirrors _solve_sym4_np below; keep in sync."""
    def wm(t):
        return W[:, _mcol(t) : _mcol(t) + 1]

    def sc(i):
        return S[:, i : i + 1]

    a, bb, cc, dd = wm(2), wm(3), wm(4), wm(5)       # read-only
    e, f, h, gg, i_, jj = (sc(k) for k in range(6))  # overwritten
    r0, r1, r2, r3 = (sc(k) for k in range(6, 10))
    p0, p1, p2, p3 = (X[:, k : k + 1] for k in range(4))
    l1, l2, l3 = (X[:, k : k + 1] for k in range(4, 7))
    t0 = X[:, 7:8]
    y0, y1, y2, y3 = (X[:, k : k + 1] for k in range(8, 12))

    mul = nc.vector.tensor_mul
    sub = nc.vector.tensor_sub
    rec = nc.vector.reciprocal

    def upd(x, l, src):  # x -= l*src
        mul(t0, l, src)
        sub(x, x, t0)

    rec(p0, a)
    mul(l1, bb, p0); mul(l2, cc, p0); mul(l3, dd, p0)
    upd(e, l1, bb); upd(f, l2, bb); upd(gg, l3, bb)
    upd(h, l2, cc); upd(i_, l3, cc); upd(jj, l3, dd)
    upd(r1, l1, r0); upd(r2, l2, r0); upd(r3, l3, r0)

    rec(p1, e)
    mul(l2, f, p1); mul(l3, gg, p1)
    upd(h, l2, f); upd(i_, l3, f); upd(jj, l3, gg)
    upd(r2, l2, r1); upd(r3, l3, r1)

    rec(p2, h)
    mul(l3, i_, p2)
    upd(jj, l3, i_); upd(r3, l3, r2)

    rec(p3, jj)
    mul(y3, r3, p3)
    upd(r2, i_, y3); mul(y2, r2, p2)
    upd(r1, f, y2); upd(r1, gg, y3); mul(y1, r1, p1)
    upd(r0, bb, y1); upd(r0, cc, y2); upd(r0, dd, y3); mul(y0, r0, p0)
    return [y0, y1, y2, y3]


# ---------------------------------------------------------------------------
# numpy mirror (for verification without hardware)

def _bf(x):
    return np.asarray(x).astype(ml_dtypes.bfloat16).astype(np.float32)


def _solve_sym4_np(m):
    """m: [n, 9] float32 (m[:, t] = m_t, col 0 unused). Returns y [n, 4]."""
    col = lambda t: m[:, t : t + 1].astype(np.float32)
    a, bb, cc, dd = col(2), col(3), col(4), col(5)
    e, f, h, gg, i_, jj = col(4), col(5), col(6), col(6), col(7), col(8)
    r0, r1, r2, r3 = col(1), col(2), col(3), col(4)
    p0 = np.float32(1.0) / a
    l1, l2, l3 = bb * p0, cc * p0, dd * p0
    e = e - l1 * bb; f = f - l2 * bb; gg = gg - l3 * bb
    h = h - l2 * cc; i_ = i_ - l3 * cc; jj = jj - l3 * dd
    r1 = r1 - l1 * r0; r2 = r2 - l2 * r0; r3 = r3 - l3 * r0
    p1 = np.float32(1.0) / e
    l2, l3 = f * p1, gg * p1
    h = h - l2 * f; i_ = i_ - l3 * f; jj = jj - l3 * gg
    r2 = r2 - l2 * r1; r3 = r3 - l3 * r1
    p2 = np.float32(1.0) / h
    l3 = i_ * p2
    jj = jj - l3 * i_; r3 = r3 - l3 * r2
    p3 = np.float32(1.0) / jj
    y3 = r3 * p3
    r2 = r2 - i_ * y3; y2 = r2 * p2
    r1 = r1 - f * y2; r1 = r1 - gg * y3; y1 = r1 * p1
    r0 = r0 - bb * y1; r0 = r0 - cc * y2; r0 = r0 - dd * y3; y0 = r0 * p0
    return np.concatenate([y0, y1, y2, y3], axis=1)


def _mirror_numpy(D, P, R):
    """Bit-approximate mirror of the device algorithm (bf16 rounding at the
    same points), for offline validation."""
    Db, Pb, Rb = _bf(D), _bf(P), _bf(R)
    b = D.shape[0]
    C = [None] * (RANK + 1)
    C[0] = _bf(Db - Pb)
    for i in range(1, RANK + 1):
        T = _bf(np.einsum("bij,bjk->bik", C[i - 1], Rb, dtype=np.float32))
        C[i] = _bf(np.einsum("bij,bjk->bik", Rb, T, dtype=np.float32))
    mu = np.zeros((b, 9), dtype=np.float32)
    for t, (i, j) in enumerate(MUPAIR):
        mu[:, t] = np.sum(C[i].astype(np.float32) * C[j].astype(np.float32),
                          axis=(1, 2))
    # forward differences
    V = mu.copy()
    m = np.zeros((b, 9), dtype=np.float32)
    for t in range(1, 9):
        V = (V[:, 1:] - V[:, :-1]).astype(np.float32)
        m[:, t] = V[:, 0]
    y = _solve_sym4_np(m)
    y0, y1, y2, y3 = (y[:, k : k + 1] for k in range(4))
    d3 = -y3
    d2 = 3 * y3 - y2
    d1 = 2 * y2 - y1 - 3 * y3
    d0 = y1 - y0 + y3 - y2
    dl = _bf(np.concatenate([d0, d1, d2, d3], axis=1))
    # PE combo: (delta_k I)_bf16 @ C_k accumulated in fp32 PSUM
    return sum(dl[:, k, None, None] * C[k] for k in range(4)).astype(np.float32)


# ---------------------------------------------------------------------------

_NC_CACHE = None


def _get_nc():
    global _NC_CACHE
    if _NC_CACHE is None:
        _NC_CACHE = build_core_kernel()
    return _NC_CACHE


def kernel(D, P, R, max_rank=4, _trace=False):
    BF = ml_dtypes.bfloat16
    D = np.ascontiguousarray(np.asarray(D, dtype=np.float32).astype(BF))
    P = np.ascontiguousarray(np.asarray(P, dtype=np.float32).astype(BF))
    R = np.ascontiguousarray(np.asarray(R, dtype=np.float32).astype(BF))
    nc = _get_nc()
    in_maps = []
    for i in range(NCORES):
        sl = slice(i * MPC, (i + 1) * MPC)
        in_maps.append({"D": D[sl], "P": P[sl], "Rm": R[sl]})
    res = run_bass_kernel_spmd(nc, in_maps, core_ids=list(range(NCORES)),
                               trace=_trace)
    out = np.concatenate([r["OUT"] for r in res.results], axis=0)
    if _trace:
        kernel.last_exec_time_ns = res.exec_time_ns
        kernel.last_trace = res.instructions_and_trace
    return out


if __name__ == "__main__":
    # offline mirror check against the jax reference
    sys.path.insert(0, "/root/problem")
    import jax

    jax.config.update("jax_platforms", "cpu")
    import reference

    inputs = {k: np.asarray(v) for k, v in reference.setup_inputs().items()}
    expected = np.asarray(reference.reference(**reference.setup_inputs()))
    got = _mirror_numpy(inputs["D"], inputs["P"], inputs["R"])
    scale = np.abs(expected).max()
    rel = np.abs(got - expected).max() / scale
    print(f"mirror rel err: {rel:.3e} (scale {scale:.3f})")
